# revision 10
# baseline (speedup 1.0000x reference)
"""GCGRU cell (graph-conv GRU, diffusion-conv gates) on 8 TRN2 NeuronCores.

Math (per batch b, N=1024 nodes, D=2 in-feats, U=64 units, S=2 supports):
  x0   = [H_b | inputs_b]                          (N, 66)  (feature-permuted)
  for gate g in {r, u, c}:
    pre_g = x0g @ Wg_m0 + sum_s A_s @ (x0g @ Wg_{m=s+1}) + bias_g
  (reassociated: (A_s @ x0) @ W == A_s @ (x0 @ W), so the N x N supports
   multiply a tiny (N, 64) matrix instead of the other association order)
  r, u = sigmoid(pre_r), sigmoid(pre_u); c = tanh(pre_c with x0c=[r*H|inputs])
  h = u * H + (1 - u) * c

Implementation notes:
  - Data parallel over batch: 32 batches -> 4 per core, no collectives.
  - supports[b] is cast f32->bf16 *during* the HBM->SBUF DMA (SWDGE cast)
    in natural layout (i on partitions, (j,s) free).
  - The j-contraction needs j on partitions, so A is transposed 128x128
    tile-wise on the TensorEngine (transpose-mode matmul with a bf16
    identity); the bf16 PSUM tiles drain on the DVE with an fp8e4 cast,
    so A^T lives in SBUF as fp8 (halving its footprint).
  - r/u pre-activations accumulate with fp8 DoubleRow matmuls (two
    j-blocks per instruction, Z_ru also fp8); the c pass runs mixed
    (fp8 A^T moving x bf16 Z_c stationary).  Tolerance is 2e-2 and the
    fp8 path lands ~1.7e-2 (validated against the fp32 reference).
  - h_prev/inputs load contiguously (2KB runs, in-DMA bf16 cast) in a
    node%8-interleaved partition layout; the PE transposes it and the
    PSUM drain un-interleaves with a strided write AP.
  - The supports chunk loads are issued FIRST (ahead of h/x) and batch
    0's first chunk is split so the PE starts transposing ~1.5us in.
    Gate elementwise runs on the DVE/Scalar so the GpSimd queue carries
    only SWDGE descriptor generation and never blocks the A stream.
"""

import numpy as np

import concourse.bacc as bacc
import concourse.mybir as mybir
import concourse.tile as tile
from concourse.bass_utils import run_bass_kernel_spmd
from concourse.masks import make_identity

B, N, D, U, S = 32, 1024, 2, 64, 2
F = D + U                      # 66
NCORES = 8
BPC = B // NCORES              # 4 batches per core
P = 128                        # partitions
JB = N // P                    # 8 j-blocks per support
K8 = N // P                    # 8 nodes per partition in contiguous layout
F32 = mybir.dt.float32
BF16 = mybir.dt.bfloat16
F8 = mybir.dt.float8e4

FP8 = True                     # fp8e4 A^T / Z_ru fast path

_COMPILED = {}


def _build():
    nc = bacc.Bacc("TRN2", target_bir_lowering=False, debug=False)

    t_inputs = nc.dram_tensor("inputs", [BPC, N, D], F32, kind="ExternalInput")
    t_supports = nc.dram_tensor("supports", [BPC, N, N, S], F32, kind="ExternalInput")
    t_hprev = nc.dram_tensor("h_prev", [BPC, N * U], F32, kind="ExternalInput")
    t_wk = {g: nc.dram_tensor(f"{g}_kernel", [F * 3, U], F32, kind="ExternalInput")
            for g in "ruc"}
    t_wb = {g: nc.dram_tensor(f"{g}_bias", [U], F32, kind="ExternalInput")
            for g in "ruc"}
    t_out = nc.dram_tensor("out", [BPC, N * U], F32, kind="ExternalOutput")

    QC = 2                 # i-tiles per load chunk
    NCH = N // (QC * P)    # 4 chunks per batch
    NQ = 512               # phase moving-slice width (one PSUM bank of f32)
    NIC = N // NQ          # 2 phase column-groups per batch
    AT_DT = F8 if FP8 else BF16
    AB_DT = F8 if FP8 else BF16
    DR = mybir.MatmulPerfMode.DoubleRow

    with tile.TileContext(nc) as tc:
        with (
            tc.tile_pool(name="const", bufs=1) as constp,
            tc.tile_pool(name="wt", bufs=1) as wtp,
            tc.tile_pool(name="pre", bufs=BPC) as prep,
            tc.tile_pool(name="abf", bufs=8) as abfp,
            tc.tile_pool(name="at", bufs=2) as atp,
            tc.tile_pool(name="act", bufs=2) as actp,
            tc.tile_pool(name="psT", bufs=4, space="PSUM") as psT,
            tc.tile_pool(name="psB", bufs=2, space="PSUM") as psB,
            tc.tile_pool(name="psM", bufs=2, space="PSUM") as psM,
        ):
            sup4 = t_supports.ap().rearrange(
                "b (p q) j two -> b p q (j two)", q=K8)

            # ---- constants ----
            id_bf = constp.tile([P, P], BF16, tag="id_bf")
            make_identity(nc, id_bf[:])
            if FP8:
                id_ab = constp.tile([P, P], F8, tag="id_ab")
                nc.vector.tensor_copy(id_ab[:], id_bf[:])
            else:
                id_ab = id_bf

            # ---- per-batch state for the staged main loop ----
            state = {}

            def issue_loads(b, chs, split=False):
                if b not in state:
                    at = [atp.tile([P, JB * N], AT_DT, tag=f"at{s}",
                                   name=f"at{s}") for s in range(S)]
                    state[b] = {"at": at, "abts": [None] * NCH}
                for ch in chs:
                    ab = abfp.tile([P, QC * N * S], AB_DT, tag="abf", name="ab")
                    if split:
                        ab4 = ab[:].rearrange("p (q m) -> p q m", q=QC)
                        for q in range(QC):
                            nc.gpsimd.dma_start(
                                ab4[:, q, :], sup4[b, :, ch * QC + q, :])
                    else:
                        nc.gpsimd.dma_start(
                            ab[:], sup4[b, :, ch * QC:(ch + 1) * QC, :])
                    state[b]["abts"][ch] = ab

            # first chunk of the whole kernel goes out before h/x so the
            # PE has transpose work as early as possible
            issue_loads(0, [0], split=True)

            # ---- gate weights, hop blocks, permuted to [H|inputs], bf16 ----
            # W rows are (f, m) pairs, m fastest: row f*3 + m.  One staging
            # DMA per gate (rows permuted to [H|inputs]), bf16 casts on DVE.
            wst = {}
            for g in "ruc":
                st = wtp.tile([F, 3 * U], F32, tag=f"wst_{g}", name=f"wst_{g}")
                src = t_wk[g].ap().rearrange("(f three) u -> f (three u)", three=3)
                nc.sync.dma_start(st[0:U, :], src[D:F, :])
                nc.sync.dma_start(st[U:F, :], src[0:D, :])
                wst[g] = st

            def w_block(g, m):
                return wst[g][:, m * U:(m + 1) * U]

            w0ru = wtp.tile([F, 2 * U], BF16, tag="w0ru")
            nc.vector.tensor_copy(w0ru[:, 0:U], w_block("r", 0))
            nc.vector.tensor_copy(w0ru[:, U:2 * U], w_block("u", 0))
            wru_s = []
            for s in range(S):
                w = wtp.tile([F, 2 * U], BF16, tag=f"wru{s}")
                nc.vector.tensor_copy(w[:, 0:U], w_block("r", s + 1))
                nc.vector.tensor_copy(w[:, U:2 * U], w_block("u", s + 1))
                wru_s.append(w)
            wc0 = wtp.tile([F, U], BF16, tag="wc0")
            nc.vector.tensor_copy(wc0[:], w_block("c", 0))
            wc_s = []
            for s in range(S):
                w = wtp.tile([F, U], BF16, tag=f"wcs{s}")
                nc.vector.tensor_copy(w[:], w_block("c", s + 1))
                wc_s.append(w)

            bias = {}
            for g in "ruc":
                bt = wtp.tile([U, 1], F32, tag=f"bias_{g}")
                nc.sync.dma_start(bt[:], t_wb[g].ap().rearrange("(u one) -> u one", one=1))
                bias[g] = bt

            # ---- prologue: x0^T and Z_ru for ALL batches ----
            # h_prev/inputs load contiguously (node n = 8p+k on partition p,
            # slot k) with in-DMA bf16 cast; 16 transpose-mode matmuls per
            # batch land [H^T | x^T] interleaved in one PSUM bank; the
            # scalar drain un-interleaves via a strided write AP.
            hcall = prep.tile([P, BPC * K8 * U], BF16, tag="hcb", name="hcb",
                              bufs=1)
            nc.gpsimd.dma_start(
                hcall[:].rearrange("p (b m) -> p b m", b=BPC),
                t_hprev.ap().rearrange("b (p m) -> p b m", p=P))
            xcall = prep.tile([P, BPC * K8 * D], BF16, tag="xcb", name="xcb",
                              bufs=1)
            nc.gpsimd.dma_start(
                xcall[:].rearrange("p (b m) -> p b m", b=BPC),
                t_inputs.ap().rearrange("b (p k) d -> p b (k d)", p=P))
            hcb = [hcall[:, b * K8 * U:(b + 1) * K8 * U] for b in range(BPC)]
            xcb = [xcall[:, b * K8 * D:(b + 1) * K8 * D] for b in range(BPC)]

            # rest of batch 0's supports go out right behind h/x
            issue_loads(0, [1, 2, 3])

            x0Tb_l, zru_l = [], []

            def prologue(b):
                px = psM.tile([F, JB * P], BF16, tag="psM", name="px")
                for k in range(K8):
                    nc.tensor.matmul(
                        px[0:U, k * P:(k + 1) * P],
                        hcb[b][:, k * U:(k + 1) * U],
                        id_bf[:], start=(k == 0), stop=False,
                        is_transpose=True)
                for k in range(K8):
                    nc.tensor.matmul(
                        px[U:F, k * P:(k + 1) * P],
                        xcb[b][:, k * D:(k + 1) * D],
                        id_bf[:], start=False, stop=(k == K8 - 1),
                        is_transpose=True)
                x0Tb = prep.tile([F, N], BF16, tag="x0Tb", name="x0Tb")
                nc.scalar.copy(x0Tb[:], px[:])
                x0Tb_l.append(x0Tb)

                zru = []
                for s in range(S):
                    z = prep.tile([P, JB * 2 * U], AT_DT, tag=f"zru{s}",
                                  name=f"zru{s}")
                    for hf in range(2):
                        pz = psM.tile([P, 4 * 2 * U], F32, tag="psM",
                                      name="pz")
                        for q in range(4):
                            nc.tensor.matmul(
                                pz[:, q * 2 * U:(q + 1) * 2 * U],
                                x0Tb[:, (4 * hf + q) * P:(4 * hf + q + 1) * P],
                                wru_s[s][:], start=(q == 0), stop=(q == 3))
                        nc.scalar.copy(
                            z[:, hf * 8 * U:(hf + 1) * 8 * U], pz[:])
                    zru.append(z)
                zru_l.append(zru)

            def transpose_chunk(b, ch):
                st = state[b]
                at, abts = st["at"], st["abts"]
                ab5 = abts[ch][:].rearrange(
                    "p (q g e two) -> p q e g two", q=QC, e=K8, two=2)
                for s in range(S):
                    atv = at[s][:].rearrange("p (jb n) -> p jb n", n=N)
                    for q in range(QC):
                        if FP8:
                            # fp8 transpose-mode requires element step 2 in
                            # PSUM: allocate double-width, write every other
                            # byte, and the drain reads the strided view
                            pt2 = psT.tile([P, 2 * JB * P], AB_DT, tag="psT",
                                           name="pt")
                            pt = pt2[:].rearrange(
                                "p (n two) -> p n two", two=2)[:, :, 0]
                        else:
                            pt = psT.tile([P, JB * P], AB_DT, tag="psT",
                                          name="pt")[:]
                        for e in range(JB):
                            nc.tensor.matmul(
                                pt[:, e * P:(e + 1) * P],
                                ab5[:, q, e, :, s],
                                id_ab[:],
                                start=(e == 0), stop=(e == JB - 1),
                                is_transpose=True)
                        c0 = (ch * QC + q) * P
                        nc.vector.tensor_copy(
                            atv[:, :, c0:c0 + P],
                            pt.rearrange("p (jb q) -> p jb q", q=P))

            def prepare_phase1(b):
                st = state[b]
                at = st["at"]
                x0Tb, zru = x0Tb_l[b], zru_l[b]

                rT = actp.tile([U, N], BF16, tag="rT", name="rT")
                uT = actp.tile([U, N], F32, tag="uT", name="uT")

                def phase1(ic):
                    p1 = psB.tile([P, NQ], F32, tag="psB", name="p1")
                    k = 0
                    if FP8:
                        for s in range(S):
                            atv = at[s][:].rearrange(
                                "p (jb n) -> p jb n", n=N)
                            zrv = zru[s][:].rearrange(
                                "p (jb m) -> p jb m", m=2 * U)
                            for jp in range(JB // 2):
                                nc.tensor.matmul(
                                    p1[:],
                                    zrv[:, 2 * jp:2 * jp + 2, :],
                                    atv[:, 2 * jp:2 * jp + 2,
                                        ic * NQ:(ic + 1) * NQ],
                                    start=(k == 0), stop=False,
                                    perf_mode=DR, skip_group_check=True)
                                k += 1
                    else:
                        for s in range(S):
                            for jb in range(JB):
                                nc.tensor.matmul(
                                    p1[:],
                                    zru[s][:, jb * 2 * U:(jb + 1) * 2 * U],
                                    at[s][:, jb * N + ic * NQ: jb * N + (ic + 1) * NQ],
                                    start=(k == 0), stop=False,
                                    skip_group_check=True)
                                k += 1
                    nc.tensor.matmul(
                        p1[:], w0ru[:], x0Tb[:, ic * NQ:(ic + 1) * NQ],
                        start=False, stop=True, skip_group_check=True)
                    nc.scalar.activation(
                        rT[:, ic * NQ:(ic + 1) * NQ], p1[0:U, :],
                        mybir.ActivationFunctionType.Sigmoid, bias=bias["r"][:])
                    nc.scalar.activation(
                        uT[:, ic * NQ:(ic + 1) * NQ], p1[U:2 * U, :],
                        mybir.ActivationFunctionType.Sigmoid, bias=bias["u"][:])

                x0cT = actp.tile([F, N], BF16, tag="x0cT", name="x0cT")
                zc = [actp.tile([P, JB * U], BF16, tag=f"zc{s}",
                                name=f"zc{s}") for s in range(S)]

                def zc_half(hf):
                    # Z_c_s = x0c @ Wc_{s+1} for node blocks of one column
                    # half; x0c^T = [(r*H)^T | x^T] from this half's r
                    sl = slice(hf * NQ, (hf + 1) * NQ)
                    if hf == 0:
                        nc.vector.tensor_copy(x0cT[U:F, :], x0Tb[U:F, :])
                    nc.vector.tensor_mul(
                        x0cT[0:U, sl], rT[:, sl], x0Tb[0:U, sl])
                    for s in range(S):
                        pz = psM.tile([P, 8 * U], F32, tag="psM", name="pzc")
                        for q in range(4 * hf, 4 * hf + 4):
                            nc.tensor.matmul(
                                pz[:, (q - 4 * hf) * U:(q - 4 * hf + 1) * U],
                                x0cT[:, q * P:(q + 1) * P],
                                wc_s[s][:], start=(q == 4 * hf),
                                stop=(q == 4 * hf + 3))
                        nc.scalar.copy(
                            zc[s][:, hf * 4 * U:(hf + 1) * 4 * U],
                            pz[:, 0:4 * U])

                def finish_ru():
                    # g1 = u*H^T and w = 1-u as soon as u is complete; they
                    # are DVE ops and unblock the tail's h math
                    g1 = actp.tile([U, N], F32, tag="g1", name="g1")
                    nc.vector.tensor_mul(g1[:], uT[:], x0Tb[0:U, :])
                    wT = actp.tile([U, N], F32, tag="wT", name="wT")
                    nc.vector.tensor_scalar(wT[:], uT[:], -1.0, 1.0,
                                            mybir.AluOpType.mult,
                                            mybir.AluOpType.add)
                    st["g1"], st["wT"] = g1, wT

                st["rT"], st["uT"] = rT, uT
                st["phase1"], st["finish_ru"] = phase1, finish_ru
                st["x0cT"], st["zc"], st["zc_half"] = x0cT, zc, zc_half

            def tail(b):
                st = state[b]
                at = st["at"]
                x0cT, zc = st["x0cT"], st["zc"]
                g1, wT = st["g1"], st["wT"]
                x0Tb = x0Tb_l[b]

                # phase 2 + h, pipelined per column half; the j-blocks of
                # the first zc half accumulate first so the second half's
                # zc matmuls overlap the p2 accumulation
                def p2_half(p2, ic, hf, k0):
                    k = k0
                    for s in range(S):
                        for jb in range(4 * hf, 4 * hf + 4):
                            nc.tensor.matmul(
                                p2[:],
                                zc[s][:, jb * U:(jb + 1) * U],
                                at[s][:, jb * N + ic * NQ: jb * N + (ic + 1) * NQ],
                                start=(k == 0), stop=False,
                                skip_group_check=True)
                            k += 1
                    return k

                cT = actp.tile([U, N], F32, tag="cT", name="cT")
                hTb = actp.tile([U, N], BF16, tag="hTb", name="hTb")
                hnat = actp.tile([P, JB * U], F32, tag="hnat", name="hnat")
                p2s = [psB.tile([U, NQ], F32, tag="psB", name=f"p2_{ic}")
                       for ic in range(NIC)]
                # half0 of ic=0 first (zc half0 ready), then the second zc
                # half (gpsimd-free: DVE mul + PE matmuls), then the rest
                k00 = p2_half(p2s[0], 0, 0, 0)
                st["zc_half"](1)
                for ic in range(NIC):
                    p2 = p2s[ic]
                    if ic == 0:
                        k = p2_half(p2, ic, 1, k00)
                    else:
                        k = p2_half(p2, ic, 0, 0)
                        k = p2_half(p2, ic, 1, k)
                    nc.tensor.matmul(
                        p2[:], wc0[:], x0cT[:, ic * NQ:(ic + 1) * NQ],
                        start=False, stop=True, skip_group_check=True)
                    sl = slice(ic * NQ, (ic + 1) * NQ)
                    nc.scalar.activation(
                        cT[:, sl], p2[:],
                        mybir.ActivationFunctionType.Tanh, bias=bias["c"][:])
                    # h^T = c^T * (1-u^T) + u^T * H^T
                    nc.vector.tensor_mul(cT[:, sl], cT[:, sl], wT[:, sl])
                    nc.vector.tensor_add(hTb[:, sl], cT[:, sl], g1[:, sl])
                    ph = psM.tile([P, 2 * JB * U], BF16, tag="psM", name="ph")
                    for jb in range(4 * ic, 4 * ic + 4):
                        nc.tensor.matmul(
                            ph[:, (jb - 4 * ic) * U:(jb - 4 * ic + 1) * U],
                            hTb[:, jb * P:(jb + 1) * P],
                            id_bf[0:U, 0:U],
                            start=(jb == 4 * ic), stop=(jb == 4 * ic + 3),
                            is_transpose=True)
                    hh = slice(ic * 4 * U, (ic + 1) * 4 * U)
                    nc.scalar.copy(hnat[:, hh], ph[:, 0:4 * U])
                    nc.sync.dma_start(
                        t_out.ap()[b].rearrange("(p m) -> p m", p=P)[:, hh],
                        hnat[:, hh])

            # ---- staged main loop ----
            # PE order: first chunk's transposes as soon as data lands,
            # prologues threaded between transpose chunks, loads for b+1
            # issued before the tail of b so SWDGE never waits on compute.
            # phase/tail work is interleaved between transpose chunks so a
            # not-yet-landed chunk never blocks ready work at the PE queue
            # head.
            transpose_chunk(0, 0)
            prologue(0)
            transpose_chunk(0, 1)
            prologue(1)
            transpose_chunk(0, 2)
            prologue(2)
            prepare_phase1(0)
            state[0]["phase1"](0)
            state[0]["zc_half"](0)
            transpose_chunk(0, 3)
            prologue(3)
            state[0]["phase1"](1)
            state[0]["finish_ru"]()
            for b in range(1, BPC):
                issue_loads(b, range(NCH))
                transpose_chunk(b, 0)
                transpose_chunk(b, 1)
                tail(b - 1)
                transpose_chunk(b, 2)
                prepare_phase1(b)
                state[b]["phase1"](0)
                state[b]["zc_half"](0)
                transpose_chunk(b, 3)
                state[b]["phase1"](1)
                state[b]["finish_ru"]()
            tail(BPC - 1)

    nc.finalize()
    return nc


def _make_in_maps(inputs):
    in_maps = []
    for c in range(NCORES):
        lo, hi = c * BPC, (c + 1) * BPC
        in_maps.append({
            "inputs": np.ascontiguousarray(inputs["inputs"][lo:hi], np.float32),
            "supports": np.ascontiguousarray(inputs["supports"][lo:hi], np.float32),
            "h_prev": np.ascontiguousarray(inputs["h_prev"][lo:hi], np.float32),
            "r_kernel": np.ascontiguousarray(inputs["r_kernel"], np.float32),
            "u_kernel": np.ascontiguousarray(inputs["u_kernel"], np.float32),
            "c_kernel": np.ascontiguousarray(inputs["c_kernel"], np.float32),
            "r_bias": np.ascontiguousarray(inputs["r_bias"], np.float32),
            "u_bias": np.ascontiguousarray(inputs["u_bias"], np.float32),
            "c_bias": np.ascontiguousarray(inputs["c_bias"], np.float32),
        })
    return in_maps


def kernel(**inputs):
    nc = _COMPILED.get("nc")
    if nc is None:
        nc = _COMPILED["nc"] = _build()

    in_maps = _make_in_maps(inputs)
    last_err = None
    for _ in range(3):
        try:
            res = run_bass_kernel_spmd(nc, in_maps, core_ids=list(range(NCORES)))
            out = np.concatenate(
                [np.asarray(res.results[c]["out"]) for c in range(NCORES)], axis=0)
            return out.astype(np.float32)
        except Exception as e:  # sporadic NRT_EXEC_UNIT_UNRECOVERABLE flakes
            last_err = e
    raise last_err


# revision 16
# speedup vs baseline: 1.0341x; 1.0341x over previous
"""GCGRU cell (graph-conv GRU, diffusion-conv gates) on 8 TRN2 NeuronCores.

Math (per batch b, N=1024 nodes, D=2 in-feats, U=64 units, S=2 supports):
  x0   = [H_b | inputs_b]                          (N, 66)  (feature-permuted)
  for gate g in {r, u, c}:
    pre_g = x0g @ Wg_m0 + sum_s A_s @ (x0g @ Wg_{m=s+1}) + bias_g
  (reassociated: (A_s @ x0) @ W == A_s @ (x0 @ W), so the N x N supports
   multiply a tiny (N, 64) matrix instead of the other association order)
  r, u = sigmoid(pre_r), sigmoid(pre_u); c = tanh(pre_c with x0c=[r*H|inputs])
  h = u * H + (1 - u) * c

Implementation notes:
  - Data parallel over batch: 32 batches -> 4 per core, no collectives.
  - supports[b] is cast f32->bf16 *during* the HBM->SBUF DMA (SWDGE cast)
    in natural layout (i on partitions, (j,s) free).
  - The j-contraction needs j on partitions, so A is transposed 128x128
    tile-wise on the TensorEngine (transpose-mode matmul with a bf16
    identity); the bf16 PSUM tiles drain on the DVE with an fp8e4 cast,
    so A^T lives in SBUF as fp8 (halving its footprint).
  - r/u pre-activations accumulate with fp8 DoubleRow matmuls (two
    j-blocks per instruction, Z_ru also fp8); the c pass runs mixed
    (fp8 A^T moving x bf16 Z_c stationary).  Tolerance is 2e-2 and the
    fp8 path lands ~1.7e-2 (validated against the fp32 reference).
  - h_prev/inputs load contiguously (2KB runs, in-DMA bf16 cast) in a
    node%8-interleaved partition layout; the PE transposes it and the
    PSUM drain un-interleaves with a strided write AP.
  - The supports chunk loads are issued FIRST (ahead of h/x) and batch
    0's first chunk is split so the PE starts transposing ~1.5us in.
    Gate elementwise runs on the DVE/Scalar so the GpSimd queue carries
    only SWDGE descriptor generation and never blocks the A stream.
"""

import numpy as np

import concourse.bacc as bacc
import concourse.mybir as mybir
import concourse.tile as tile
from concourse.bass_utils import run_bass_kernel_spmd
from concourse.masks import make_identity

B, N, D, U, S = 32, 1024, 2, 64, 2
F = D + U                      # 66
NCORES = 8
BPC = B // NCORES              # 4 batches per core
P = 128                        # partitions
JB = N // P                    # 8 j-blocks per support
K8 = N // P                    # 8 nodes per partition in contiguous layout
F32 = mybir.dt.float32
BF16 = mybir.dt.bfloat16
F8 = mybir.dt.float8e4

FP8 = True                     # fp8e4 A^T / Z_ru fast path

_COMPILED = {}


def _build():
    nc = bacc.Bacc("TRN2", target_bir_lowering=False, debug=False)

    t_inputs = nc.dram_tensor("inputs", [BPC, N, D], F32, kind="ExternalInput")
    t_supports = nc.dram_tensor("supports", [BPC, N, N, S], F32, kind="ExternalInput")
    t_hprev = nc.dram_tensor("h_prev", [BPC, N * U], F32, kind="ExternalInput")
    t_wk = {g: nc.dram_tensor(f"{g}_kernel", [F * 3, U], F32, kind="ExternalInput")
            for g in "ruc"}
    t_wb = {g: nc.dram_tensor(f"{g}_bias", [U], F32, kind="ExternalInput")
            for g in "ruc"}
    t_out = nc.dram_tensor("out", [BPC, N * U], F32, kind="ExternalOutput")

    QC = 2                 # i-tiles per load chunk
    NCH = N // (QC * P)    # 4 chunks per batch
    NQ = 512               # phase moving-slice width (one PSUM bank of f32)
    NIC = N // NQ          # 2 phase column-groups per batch
    AT_DT = F8 if FP8 else BF16
    AB_DT = F8 if FP8 else BF16
    DR = mybir.MatmulPerfMode.DoubleRow

    with tile.TileContext(nc) as tc:
        with (
            tc.tile_pool(name="const", bufs=1) as constp,
            tc.tile_pool(name="wt", bufs=1) as wtp,
            tc.tile_pool(name="pre", bufs=BPC) as prep,
            tc.tile_pool(name="abf", bufs=8) as abfp,
            tc.tile_pool(name="at", bufs=2) as atp,
            tc.tile_pool(name="act", bufs=2) as actp,
            tc.tile_pool(name="psT", bufs=2, space="PSUM") as psT,
            tc.tile_pool(name="psB", bufs=2, space="PSUM") as psB,
            tc.tile_pool(name="psM", bufs=2, space="PSUM") as psM,
        ):
            sup4 = t_supports.ap().rearrange(
                "b (p q) j two -> b p q (j two)", q=K8)

            # ---- constants ----
            id_bf = constp.tile([P, P], BF16, tag="id_bf")
            make_identity(nc, id_bf[:])
            if FP8:
                id_ab = constp.tile([P, P], F8, tag="id_ab")
                nc.vector.tensor_copy(id_ab[:], id_bf[:])
            else:
                id_ab = id_bf

            # ---- per-batch state for the staged main loop ----
            state = {}

            def issue_loads(b, chs, split=False):
                if b not in state:
                    at = [atp.tile([P, JB * N], AT_DT, tag=f"at{s}",
                                   name=f"at{s}") for s in range(S)]
                    state[b] = {"at": at, "abts": [None] * NCH}
                for ch in chs:
                    ab = abfp.tile([P, QC * N * S], AB_DT, tag="abf", name="ab")
                    if split:
                        ab4 = ab[:].rearrange("p (q m) -> p q m", q=QC)
                        for q in range(QC):
                            nc.gpsimd.dma_start(
                                ab4[:, q, :], sup4[b, :, ch * QC + q, :])
                    else:
                        nc.gpsimd.dma_start(
                            ab[:], sup4[b, :, ch * QC:(ch + 1) * QC, :])
                    state[b]["abts"][ch] = ab

            # first chunk of the whole kernel goes out before h/x so the
            # PE has transpose work as early as possible
            issue_loads(0, [0], split=True)

            # ---- gate weights, hop blocks, permuted to [H|inputs], bf16 ----
            # W rows are (f, m) pairs, m fastest: row f*3 + m.  One staging
            # DMA per gate (rows permuted to [H|inputs]), bf16 casts on DVE.
            wst = {}
            for g in "ruc":
                st = wtp.tile([F, 3 * U], F32, tag=f"wst_{g}", name=f"wst_{g}")
                src = t_wk[g].ap().rearrange("(f three) u -> f (three u)", three=3)
                nc.sync.dma_start(st[0:U, :], src[D:F, :])
                nc.sync.dma_start(st[U:F, :], src[0:D, :])
                wst[g] = st

            def w_block(g, m):
                return wst[g][:, m * U:(m + 1) * U]

            w0ru = wtp.tile([F, 2 * U], BF16, tag="w0ru")
            nc.vector.tensor_copy(w0ru[:, 0:U], w_block("r", 0))
            nc.vector.tensor_copy(w0ru[:, U:2 * U], w_block("u", 0))
            wru_s = []
            for s in range(S):
                w = wtp.tile([F, 2 * U], BF16, tag=f"wru{s}")
                nc.vector.tensor_copy(w[:, 0:U], w_block("r", s + 1))
                nc.vector.tensor_copy(w[:, U:2 * U], w_block("u", s + 1))
                wru_s.append(w)
            wc0 = wtp.tile([F, U], BF16, tag="wc0")
            nc.vector.tensor_copy(wc0[:], w_block("c", 0))
            wc_s = []
            for s in range(S):
                w = wtp.tile([F, U], BF16, tag=f"wcs{s}")
                nc.vector.tensor_copy(w[:], w_block("c", s + 1))
                wc_s.append(w)

            bias = {}
            for g in "ruc":
                bt = wtp.tile([U, 1], F32, tag=f"bias_{g}")
                nc.sync.dma_start(bt[:], t_wb[g].ap().rearrange("(u one) -> u one", one=1))
                bias[g] = bt

            # ---- prologue: x0^T and Z_ru for ALL batches ----
            # h_prev/inputs load contiguously (node n = 8p+k on partition p,
            # slot k) with in-DMA bf16 cast; 16 transpose-mode matmuls per
            # batch land [H^T | x^T] interleaved in one PSUM bank; the
            # scalar drain un-interleaves via a strided write AP.
            hcall = prep.tile([P, BPC * K8 * U], BF16, tag="hcb", name="hcb",
                              bufs=1)
            nc.gpsimd.dma_start(
                hcall[:].rearrange("p (b m) -> p b m", b=BPC),
                t_hprev.ap().rearrange("b (p m) -> p b m", p=P))
            xcall = prep.tile([P, BPC * K8 * D], BF16, tag="xcb", name="xcb",
                              bufs=1)
            nc.gpsimd.dma_start(
                xcall[:].rearrange("p (b m) -> p b m", b=BPC),
                t_inputs.ap().rearrange("b (p k) d -> p b (k d)", p=P))
            hcb = [hcall[:, b * K8 * U:(b + 1) * K8 * U] for b in range(BPC)]
            xcb = [xcall[:, b * K8 * D:(b + 1) * K8 * D] for b in range(BPC)]

            # rest of batch 0's supports go out right behind h/x
            issue_loads(0, [1, 2, 3])

            x0Tb_l, zru_l = [], []

            def prologue(b):
                px = psM.tile([F, JB * P], BF16, tag="psM", name="px")
                for k in range(K8):
                    nc.tensor.matmul(
                        px[0:U, k * P:(k + 1) * P],
                        hcb[b][:, k * U:(k + 1) * U],
                        id_bf[:], start=(k == 0), stop=False,
                        is_transpose=True)
                for k in range(K8):
                    nc.tensor.matmul(
                        px[U:F, k * P:(k + 1) * P],
                        xcb[b][:, k * D:(k + 1) * D],
                        id_bf[:], start=False, stop=(k == K8 - 1),
                        is_transpose=True)
                x0Tb = prep.tile([F, N], BF16, tag="x0Tb", name="x0Tb")
                nc.scalar.copy(x0Tb[:], px[:])
                x0Tb_l.append(x0Tb)

                zru = []
                for s in range(S):
                    z = prep.tile([P, JB * 2 * U], AT_DT, tag=f"zru{s}",
                                  name=f"zru{s}")
                    for hf in range(2):
                        pz = psM.tile([P, 4 * 2 * U], F32, tag="psM",
                                      name="pz")
                        for q in range(4):
                            nc.tensor.matmul(
                                pz[:, q * 2 * U:(q + 1) * 2 * U],
                                x0Tb[:, (4 * hf + q) * P:(4 * hf + q + 1) * P],
                                wru_s[s][:], start=(q == 0), stop=(q == 3))
                        nc.scalar.copy(
                            z[:, hf * 8 * U:(hf + 1) * 8 * U], pz[:])
                    zru.append(z)
                zru_l.append(zru)

            def transpose_chunk(b, ch):
                # fp8 transpose-mode requires element step 2 in PSUM:
                # allocate double-width, write every other byte, and the
                # drains read the strided view.  Both q sub-tiles of a
                # chunk share one two-bank PSUM tile so the drain is a
                # single [128, 2*128-col] copy per (s, chunk); s=0 drains
                # on the DVE and s=1 on the Scalar engine to split the
                # PSUM-recycle bandwidth.
                st = state[b]
                at, abts = st["at"], st["abts"]
                ab5 = abts[ch][:].rearrange(
                    "p (q g e two) -> p q e g two", q=QC, e=K8, two=2)
                for s in range(S):
                    atv = at[s][:].rearrange("p (jb n) -> p jb n", n=N)
                    if FP8:
                        ptb = psT.tile([P, QC * 2 * JB * P], AB_DT,
                                       tag="psT", name="pt")
                        ptq = ptb[:].rearrange(
                            "p (q n two) -> p q n two", q=QC, two=2)
                        pts = [ptq[:, q, :, 0] for q in range(QC)]
                        # drain source ordered as dest: [jb, q, i]
                        dr_src = ptb[:].rearrange(
                            "p (q jb i two) -> p jb q (i two)",
                            q=QC, jb=JB, two=2)[:, :, :, 0:2 * P:2]
                    else:
                        ptb = psT.tile([P, QC * JB * P], AB_DT, tag="psT",
                                       name="pt")
                        ptq = ptb[:].rearrange(
                            "p (q n) -> p q n", q=QC)
                        pts = [ptq[:, q, :] for q in range(QC)]
                        dr_src = ptb[:].rearrange(
                            "p (q jb i) -> p jb q i", q=QC, jb=JB)
                    for q in range(QC):
                        pt = pts[q]
                        for e in range(JB):
                            nc.tensor.matmul(
                                pt[:, e * P:(e + 1) * P],
                                ab5[:, q, e, :, s],
                                id_ab[:],
                                start=(e == 0), stop=(e == JB - 1),
                                is_transpose=True)
                    c0 = ch * QC * P
                    if s == 0:
                        nc.vector.tensor_copy(
                            atv[:, :, c0:c0 + QC * P], dr_src)
                    else:
                        nc.scalar.copy(
                            atv[:, :, c0:c0 + QC * P], dr_src)

            def prepare_phase1(b):
                st = state[b]
                at = st["at"]
                x0Tb, zru = x0Tb_l[b], zru_l[b]

                rT = actp.tile([U, N], BF16, tag="rT", name="rT")
                uT = actp.tile([U, N], F32, tag="uT", name="uT")

                def phase1(ic):
                    p1 = psB.tile([P, NQ], F32, tag="psB", name="p1")
                    k = 0
                    if FP8:
                        for s in range(S):
                            atv = at[s][:].rearrange(
                                "p (jb n) -> p jb n", n=N)
                            zrv = zru[s][:].rearrange(
                                "p (jb m) -> p jb m", m=2 * U)
                            for jp in range(JB // 2):
                                nc.tensor.matmul(
                                    p1[:],
                                    zrv[:, 2 * jp:2 * jp + 2, :],
                                    atv[:, 2 * jp:2 * jp + 2,
                                        ic * NQ:(ic + 1) * NQ],
                                    start=(k == 0), stop=False,
                                    perf_mode=DR, skip_group_check=True)
                                k += 1
                    else:
                        for s in range(S):
                            for jb in range(JB):
                                nc.tensor.matmul(
                                    p1[:],
                                    zru[s][:, jb * 2 * U:(jb + 1) * 2 * U],
                                    at[s][:, jb * N + ic * NQ: jb * N + (ic + 1) * NQ],
                                    start=(k == 0), stop=False,
                                    skip_group_check=True)
                                k += 1
                    nc.tensor.matmul(
                        p1[:], w0ru[:], x0Tb[:, ic * NQ:(ic + 1) * NQ],
                        start=False, stop=True, skip_group_check=True)
                    nc.scalar.activation(
                        rT[:, ic * NQ:(ic + 1) * NQ], p1[0:U, :],
                        mybir.ActivationFunctionType.Sigmoid, bias=bias["r"][:])
                    nc.scalar.activation(
                        uT[:, ic * NQ:(ic + 1) * NQ], p1[U:2 * U, :],
                        mybir.ActivationFunctionType.Sigmoid, bias=bias["u"][:])

                x0cT = actp.tile([F, N], BF16, tag="x0cT", name="x0cT")
                zc = [actp.tile([P, JB * U], BF16, tag=f"zc{s}",
                                name=f"zc{s}") for s in range(S)]

                def zc_half(hf):
                    # Z_c_s = x0c @ Wc_{s+1} for node blocks of one column
                    # half; x0c^T = [(r*H)^T | x^T] from this half's r
                    sl = slice(hf * NQ, (hf + 1) * NQ)
                    if hf == 0:
                        nc.vector.tensor_copy(x0cT[U:F, :], x0Tb[U:F, :])
                    nc.vector.tensor_mul(
                        x0cT[0:U, sl], rT[:, sl], x0Tb[0:U, sl])
                    for s in range(S):
                        pz = psM.tile([P, 8 * U], F32, tag="psM", name="pzc")
                        for q in range(4 * hf, 4 * hf + 4):
                            nc.tensor.matmul(
                                pz[:, (q - 4 * hf) * U:(q - 4 * hf + 1) * U],
                                x0cT[:, q * P:(q + 1) * P],
                                wc_s[s][:], start=(q == 4 * hf),
                                stop=(q == 4 * hf + 3))
                        nc.scalar.copy(
                            zc[s][:, hf * 4 * U:(hf + 1) * 4 * U],
                            pz[:, 0:4 * U])

                def finish_ru():
                    # g1 = u*H^T and w = 1-u as soon as u is complete; they
                    # are DVE ops and unblock the tail's h math
                    g1 = actp.tile([U, N], F32, tag="g1", name="g1")
                    nc.vector.tensor_mul(g1[:], uT[:], x0Tb[0:U, :])
                    wT = actp.tile([U, N], F32, tag="wT", name="wT")
                    nc.vector.tensor_scalar(wT[:], uT[:], -1.0, 1.0,
                                            mybir.AluOpType.mult,
                                            mybir.AluOpType.add)
                    st["g1"], st["wT"] = g1, wT

                st["rT"], st["uT"] = rT, uT
                st["phase1"], st["finish_ru"] = phase1, finish_ru
                st["x0cT"], st["zc"], st["zc_half"] = x0cT, zc, zc_half

            def tail(b):
                st = state[b]
                at = st["at"]
                x0cT, zc = st["x0cT"], st["zc"]
                g1, wT = st["g1"], st["wT"]
                x0Tb = x0Tb_l[b]

                # phase 2 + h, pipelined per column half; the j-blocks of
                # the first zc half accumulate first so the second half's
                # zc matmuls overlap the p2 accumulation
                def p2_half(p2, ic, hf, k0):
                    k = k0
                    for s in range(S):
                        for jb in range(4 * hf, 4 * hf + 4):
                            nc.tensor.matmul(
                                p2[:],
                                zc[s][:, jb * U:(jb + 1) * U],
                                at[s][:, jb * N + ic * NQ: jb * N + (ic + 1) * NQ],
                                start=(k == 0), stop=False,
                                skip_group_check=True)
                            k += 1
                    return k

                cT = actp.tile([U, N], F32, tag="cT", name="cT")
                hTb = actp.tile([U, N], BF16, tag="hTb", name="hTb")
                hnat = actp.tile([P, JB * U], F32, tag="hnat", name="hnat")
                p2s = [psB.tile([U, NQ], F32, tag="psB", name=f"p2_{ic}")
                       for ic in range(NIC)]
                # half0 of ic=0 first (zc half0 ready), then the second zc
                # half (gpsimd-free: DVE mul + PE matmuls), then the rest
                k00 = p2_half(p2s[0], 0, 0, 0)
                st["zc_half"](1)
                for ic in range(NIC):
                    p2 = p2s[ic]
                    if ic == 0:
                        k = p2_half(p2, ic, 1, k00)
                    else:
                        k = p2_half(p2, ic, 0, 0)
                        k = p2_half(p2, ic, 1, k)
                    nc.tensor.matmul(
                        p2[:], wc0[:], x0cT[:, ic * NQ:(ic + 1) * NQ],
                        start=False, stop=True, skip_group_check=True)
                    sl = slice(ic * NQ, (ic + 1) * NQ)
                    nc.scalar.activation(
                        cT[:, sl], p2[:],
                        mybir.ActivationFunctionType.Tanh, bias=bias["c"][:])
                    # h^T = c^T * (1-u^T) + u^T * H^T
                    nc.vector.tensor_mul(cT[:, sl], cT[:, sl], wT[:, sl])
                    nc.vector.tensor_add(hTb[:, sl], cT[:, sl], g1[:, sl])
                    ph = psM.tile([P, 2 * JB * U], BF16, tag="psM", name="ph")
                    for jb in range(4 * ic, 4 * ic + 4):
                        nc.tensor.matmul(
                            ph[:, (jb - 4 * ic) * U:(jb - 4 * ic + 1) * U],
                            hTb[:, jb * P:(jb + 1) * P],
                            id_bf[0:U, 0:U],
                            start=(jb == 4 * ic), stop=(jb == 4 * ic + 3),
                            is_transpose=True)
                    hh = slice(ic * 4 * U, (ic + 1) * 4 * U)
                    nc.scalar.copy(hnat[:, hh], ph[:, 0:4 * U])
                    nc.sync.dma_start(
                        t_out.ap()[b].rearrange("(p m) -> p m", p=P)[:, hh],
                        hnat[:, hh])

            # ---- staged main loop ----
            # PE order: first chunk's transposes as soon as data lands,
            # prologues threaded between transpose chunks, loads for b+1
            # issued before the tail of b so SWDGE never waits on compute.
            # phase/tail work is interleaved between transpose chunks so a
            # not-yet-landed chunk never blocks ready work at the PE queue
            # head.
            transpose_chunk(0, 0)
            prologue(0)
            prologue(1)
            transpose_chunk(0, 1)
            prologue(2)
            prologue(3)
            transpose_chunk(0, 2)
            prepare_phase1(0)
            state[0]["phase1"](0)
            state[0]["zc_half"](0)
            transpose_chunk(0, 3)
            state[0]["phase1"](1)
            state[0]["finish_ru"]()
            for b in range(1, BPC):
                issue_loads(b, range(NCH))
                transpose_chunk(b, 0)
                transpose_chunk(b, 1)
                tail(b - 1)
                transpose_chunk(b, 2)
                prepare_phase1(b)
                state[b]["phase1"](0)
                state[b]["zc_half"](0)
                transpose_chunk(b, 3)
                state[b]["phase1"](1)
                state[b]["finish_ru"]()
            tail(BPC - 1)

    nc.finalize()
    return nc


def _make_in_maps(inputs):
    in_maps = []
    for c in range(NCORES):
        lo, hi = c * BPC, (c + 1) * BPC
        in_maps.append({
            "inputs": np.ascontiguousarray(inputs["inputs"][lo:hi], np.float32),
            "supports": np.ascontiguousarray(inputs["supports"][lo:hi], np.float32),
            "h_prev": np.ascontiguousarray(inputs["h_prev"][lo:hi], np.float32),
            "r_kernel": np.ascontiguousarray(inputs["r_kernel"], np.float32),
            "u_kernel": np.ascontiguousarray(inputs["u_kernel"], np.float32),
            "c_kernel": np.ascontiguousarray(inputs["c_kernel"], np.float32),
            "r_bias": np.ascontiguousarray(inputs["r_bias"], np.float32),
            "u_bias": np.ascontiguousarray(inputs["u_bias"], np.float32),
            "c_bias": np.ascontiguousarray(inputs["c_bias"], np.float32),
        })
    return in_maps


def kernel(**inputs):
    nc = _COMPILED.get("nc")
    if nc is None:
        nc = _COMPILED["nc"] = _build()

    in_maps = _make_in_maps(inputs)
    last_err = None
    for _ in range(3):
        try:
            res = run_bass_kernel_spmd(nc, in_maps, core_ids=list(range(NCORES)))
            out = np.concatenate(
                [np.asarray(res.results[c]["out"]) for c in range(NCORES)], axis=0)
            return out.astype(np.float32)
        except Exception as e:  # sporadic NRT_EXEC_UNIT_UNRECOVERABLE flakes
            last_err = e
    raise last_err


# revision 18
# speedup vs baseline: 1.0488x; 1.0142x over previous
"""GCGRU cell (graph-conv GRU, diffusion-conv gates) on 8 TRN2 NeuronCores.

Math (per batch b, N=1024 nodes, D=2 in-feats, U=64 units, S=2 supports):
  x0   = [H_b | inputs_b]                          (N, 66)  (feature-permuted)
  for gate g in {r, u, c}:
    pre_g = x0g @ Wg_m0 + sum_s A_s @ (x0g @ Wg_{m=s+1}) + bias_g
  (reassociated: (A_s @ x0) @ W == A_s @ (x0 @ W), so the N x N supports
   multiply a tiny (N, 64) matrix instead of the other association order)
  r, u = sigmoid(pre_r), sigmoid(pre_u); c = tanh(pre_c with x0c=[r*H|inputs])
  h = u * H + (1 - u) * c

Implementation notes:
  - Data parallel over batch: 32 batches -> 4 per core, no collectives.
  - supports[b] is cast f32->bf16 *during* the HBM->SBUF DMA (SWDGE cast)
    in natural layout (i on partitions, (j,s) free).
  - The j-contraction needs j on partitions, so A is transposed 128x128
    tile-wise on the TensorEngine (transpose-mode matmul with a bf16
    identity); the bf16 PSUM tiles drain on the DVE with an fp8e4 cast,
    so A^T lives in SBUF as fp8 (halving its footprint).
  - r/u pre-activations accumulate with fp8 DoubleRow matmuls (two
    j-blocks per instruction, Z_ru also fp8); the c pass runs mixed
    (fp8 A^T moving x bf16 Z_c stationary).  Tolerance is 2e-2 and the
    fp8 path lands ~1.7e-2 (validated against the fp32 reference).
  - h_prev/inputs load contiguously (2KB runs, in-DMA bf16 cast) in a
    node%8-interleaved partition layout; the PE transposes it and the
    PSUM drain un-interleaves with a strided write AP.
  - The supports chunk loads are issued FIRST (ahead of h/x) and batch
    0's first chunk is split so the PE starts transposing ~1.5us in.
    Gate elementwise runs on the DVE/Scalar so the GpSimd queue carries
    only SWDGE descriptor generation and never blocks the A stream.
"""

import numpy as np

import concourse.bacc as bacc
import concourse.mybir as mybir
import concourse.tile as tile
from concourse.bass_utils import run_bass_kernel_spmd
from concourse.masks import make_identity

B, N, D, U, S = 32, 1024, 2, 64, 2
F = D + U                      # 66
NCORES = 8
BPC = B // NCORES              # 4 batches per core
P = 128                        # partitions
JB = N // P                    # 8 j-blocks per support
K8 = N // P                    # 8 nodes per partition in contiguous layout
F32 = mybir.dt.float32
BF16 = mybir.dt.bfloat16
F8 = mybir.dt.float8e4

FP8 = True                     # fp8e4 A^T / Z_ru fast path

_COMPILED = {}


def _build():
    nc = bacc.Bacc("TRN2", target_bir_lowering=False, debug=False)

    t_inputs = nc.dram_tensor("inputs", [BPC, N, D], F32, kind="ExternalInput")
    t_supports = nc.dram_tensor("supports", [BPC, N, N, S], F32, kind="ExternalInput")
    t_hprev = nc.dram_tensor("h_prev", [BPC, N * U], F32, kind="ExternalInput")
    t_wk = {g: nc.dram_tensor(f"{g}_kernel", [F * 3, U], F32, kind="ExternalInput")
            for g in "ruc"}
    t_wb = {g: nc.dram_tensor(f"{g}_bias", [U], F32, kind="ExternalInput")
            for g in "ruc"}
    t_out = nc.dram_tensor("out", [BPC, N * U], F32, kind="ExternalOutput")

    QC = 2                 # i-tiles per load chunk
    NCH = N // (QC * P)    # 4 chunks per batch
    NQ = 512               # phase moving-slice width (one PSUM bank of f32)
    NIC = N // NQ          # 2 phase column-groups per batch
    AT_DT = F8 if FP8 else BF16
    AB_DT = F8 if FP8 else BF16
    DR = mybir.MatmulPerfMode.DoubleRow

    with tile.TileContext(nc) as tc:
        with (
            tc.tile_pool(name="const", bufs=1) as constp,
            tc.tile_pool(name="wt", bufs=1) as wtp,
            tc.tile_pool(name="pre", bufs=BPC) as prep,
            tc.tile_pool(name="abf", bufs=8) as abfp,
            tc.tile_pool(name="at", bufs=2) as atp,
            tc.tile_pool(name="act", bufs=2) as actp,
            tc.tile_pool(name="psT", bufs=2, space="PSUM") as psT,
            tc.tile_pool(name="psB", bufs=2, space="PSUM") as psB,
            tc.tile_pool(name="psM", bufs=2, space="PSUM") as psM,
        ):
            sup4 = t_supports.ap().rearrange(
                "b (p q) j two -> b p q (j two)", q=K8)

            # ---- constants ----
            id_bf = constp.tile([P, P], BF16, tag="id_bf")
            make_identity(nc, id_bf[:])
            if FP8:
                id_ab = constp.tile([P, P], F8, tag="id_ab")
                nc.vector.tensor_copy(id_ab[:], id_bf[:])
            else:
                id_ab = id_bf

            # ---- per-batch state for the staged main loop ----
            state = {}

            def issue_loads(b, chs, split=False):
                if b not in state:
                    at = [atp.tile([P, JB * N], AT_DT, tag=f"at{s}",
                                   name=f"at{s}") for s in range(S)]
                    state[b] = {"at": at, "abts": [None] * NCH}
                for ch in chs:
                    ab = abfp.tile([P, QC * N * S], AB_DT, tag="abf", name="ab")
                    if split:
                        ab4 = ab[:].rearrange("p (q m) -> p q m", q=QC)
                        for q in range(QC):
                            nc.gpsimd.dma_start(
                                ab4[:, q, :], sup4[b, :, ch * QC + q, :])
                    else:
                        nc.gpsimd.dma_start(
                            ab[:], sup4[b, :, ch * QC:(ch + 1) * QC, :])
                    state[b]["abts"][ch] = ab

            # first chunk of the whole kernel goes out before h/x so the
            # PE has transpose work as early as possible
            issue_loads(0, [0], split=True)

            # ---- gate weights, hop blocks, permuted to [H|inputs], bf16 ----
            # W rows are (f, m) pairs, m fastest: row f*3 + m.  One staging
            # DMA per gate (rows permuted to [H|inputs]), bf16 casts on DVE.
            wst = {}
            for g in "ruc":
                st = wtp.tile([F, 3 * U], F32, tag=f"wst_{g}", name=f"wst_{g}")
                src = t_wk[g].ap().rearrange("(f three) u -> f (three u)", three=3)
                nc.sync.dma_start(st[0:U, :], src[D:F, :])
                nc.sync.dma_start(st[U:F, :], src[0:D, :])
                wst[g] = st

            def w_block(g, m):
                return wst[g][:, m * U:(m + 1) * U]

            w0ru = wtp.tile([F, 2 * U], BF16, tag="w0ru")
            nc.vector.tensor_copy(w0ru[:, 0:U], w_block("r", 0))
            nc.vector.tensor_copy(w0ru[:, U:2 * U], w_block("u", 0))
            wru_cat = wtp.tile([F, S * 2 * U], BF16, tag="wru_cat")
            for s in range(S):
                nc.vector.tensor_copy(
                    wru_cat[:, s * 2 * U:s * 2 * U + U], w_block("r", s + 1))
                nc.vector.tensor_copy(
                    wru_cat[:, s * 2 * U + U:(s + 1) * 2 * U],
                    w_block("u", s + 1))
            wc0 = wtp.tile([F, U], BF16, tag="wc0")
            nc.vector.tensor_copy(wc0[:], w_block("c", 0))
            wc_cat = wtp.tile([F, S * U], BF16, tag="wc_cat")
            for s in range(S):
                nc.vector.tensor_copy(
                    wc_cat[:, s * U:(s + 1) * U], w_block("c", s + 1))

            bias = {}
            for g in "ruc":
                bt = wtp.tile([U, 1], F32, tag=f"bias_{g}")
                nc.sync.dma_start(bt[:], t_wb[g].ap().rearrange("(u one) -> u one", one=1))
                bias[g] = bt

            # ---- prologue: x0^T and Z_ru for ALL batches ----
            # h_prev/inputs load contiguously (node n = 8p+k on partition p,
            # slot k) with in-DMA bf16 cast; 16 transpose-mode matmuls per
            # batch land [H^T | x^T] interleaved in one PSUM bank; the
            # scalar drain un-interleaves via a strided write AP.
            hcall = prep.tile([P, BPC * K8 * U], BF16, tag="hcb", name="hcb",
                              bufs=1)
            nc.gpsimd.dma_start(
                hcall[:].rearrange("p (b m) -> p b m", b=BPC),
                t_hprev.ap().rearrange("b (p m) -> p b m", p=P))
            xcall = prep.tile([P, BPC * K8 * D], BF16, tag="xcb", name="xcb",
                              bufs=1)
            nc.gpsimd.dma_start(
                xcall[:].rearrange("p (b m) -> p b m", b=BPC),
                t_inputs.ap().rearrange("b (p k) d -> p b (k d)", p=P))
            hcb = [hcall[:, b * K8 * U:(b + 1) * K8 * U] for b in range(BPC)]
            xcb = [xcall[:, b * K8 * D:(b + 1) * K8 * D] for b in range(BPC)]

            # rest of batch 0's supports go out right behind h/x
            issue_loads(0, [1, 2, 3])

            x0Tb_l, zru_l = [], []

            def prologue(b):
                px = psM.tile([F, JB * P], BF16, tag="psM", name="px")
                for k in range(K8):
                    nc.tensor.matmul(
                        px[0:U, k * P:(k + 1) * P],
                        hcb[b][:, k * U:(k + 1) * U],
                        id_bf[:], start=(k == 0), stop=False,
                        is_transpose=True)
                for k in range(K8):
                    nc.tensor.matmul(
                        px[U:F, k * P:(k + 1) * P],
                        xcb[b][:, k * D:(k + 1) * D],
                        id_bf[:], start=False, stop=(k == K8 - 1),
                        is_transpose=True)
                x0Tb = prep.tile([F, N], BF16, tag="x0Tb", name="x0Tb")
                nc.scalar.copy(x0Tb[:], px[:])
                x0Tb_l.append(x0Tb)

                # Z_ru for both supports in one 256-col moving sweep per
                # node block; layout (jb, s, u2) so the copy is contiguous
                z = prep.tile([P, JB * S * 2 * U], AT_DT, tag="zru",
                              name="zru")
                W2 = S * 2 * U
                for qp in range(4):
                    pz = psM.tile([P, 2 * W2], F32, tag="psM", name="pz")
                    for q in (2 * qp, 2 * qp + 1):
                        nc.tensor.matmul(
                            pz[:, (q - 2 * qp) * W2:(q - 2 * qp + 1) * W2],
                            x0Tb[:, q * P:(q + 1) * P],
                            wru_cat[:], start=(q == 2 * qp),
                            stop=(q == 2 * qp + 1))
                    nc.scalar.copy(
                        z[:, 2 * qp * W2:(2 * qp + 2) * W2], pz[:])
                zru_l.append(z)

            def transpose_chunk(b, ch):
                # fp8 transpose-mode requires element step 2 in PSUM:
                # allocate double-width, write every other byte, and the
                # drains read the strided view.  Both q sub-tiles of a
                # chunk share one two-bank PSUM tile so the drain is a
                # single [128, 2*128-col] copy per (s, chunk); s=0 drains
                # on the DVE and s=1 on the Scalar engine to split the
                # PSUM-recycle bandwidth.
                st = state[b]
                at, abts = st["at"], st["abts"]
                ab5 = abts[ch][:].rearrange(
                    "p (q g e two) -> p q e g two", q=QC, e=K8, two=2)
                for s in range(S):
                    atv = at[s][:].rearrange("p (jb n) -> p jb n", n=N)
                    if FP8:
                        ptb = psT.tile([P, QC * 2 * JB * P], AB_DT,
                                       tag="psT", name="pt")
                        ptq = ptb[:].rearrange(
                            "p (q n two) -> p q n two", q=QC, two=2)
                        pts = [ptq[:, q, :, 0] for q in range(QC)]
                        # drain source ordered as dest: [jb, q, i]
                        dr_src = ptb[:].rearrange(
                            "p (q jb i two) -> p jb q (i two)",
                            q=QC, jb=JB, two=2)[:, :, :, 0:2 * P:2]
                    else:
                        ptb = psT.tile([P, QC * JB * P], AB_DT, tag="psT",
                                       name="pt")
                        ptq = ptb[:].rearrange(
                            "p (q n) -> p q n", q=QC)
                        pts = [ptq[:, q, :] for q in range(QC)]
                        dr_src = ptb[:].rearrange(
                            "p (q jb i) -> p jb q i", q=QC, jb=JB)
                    for q in range(QC):
                        pt = pts[q]
                        for e in range(JB):
                            nc.tensor.matmul(
                                pt[:, e * P:(e + 1) * P],
                                ab5[:, q, e, :, s],
                                id_ab[:],
                                start=(e == 0), stop=(e == JB - 1),
                                is_transpose=True)
                    c0 = ch * QC * P
                    if s == 0:
                        nc.vector.tensor_copy(
                            atv[:, :, c0:c0 + QC * P], dr_src)
                    else:
                        nc.scalar.copy(
                            atv[:, :, c0:c0 + QC * P], dr_src)

            def prepare_phase1(b):
                st = state[b]
                at = st["at"]
                x0Tb, zru = x0Tb_l[b], zru_l[b]

                rT = actp.tile([U, N], BF16, tag="rT", name="rT")
                uT = actp.tile([U, N], F32, tag="uT", name="uT")

                zrv = zru[:].rearrange(
                    "p (jb s m) -> p jb s m", s=S, m=2 * U)

                def phase1(ic):
                    p1 = psB.tile([P, NQ], F32, tag="psB", name="p1")
                    k = 0
                    if FP8:
                        for s in range(S):
                            atv = at[s][:].rearrange(
                                "p (jb n) -> p jb n", n=N)
                            for jp in range(JB // 2):
                                nc.tensor.matmul(
                                    p1[:],
                                    zrv[:, 2 * jp:2 * jp + 2, s, :],
                                    atv[:, 2 * jp:2 * jp + 2,
                                        ic * NQ:(ic + 1) * NQ],
                                    start=(k == 0), stop=False,
                                    perf_mode=DR, skip_group_check=True)
                                k += 1
                    else:
                        for s in range(S):
                            for jb in range(JB):
                                nc.tensor.matmul(
                                    p1[:],
                                    zrv[:, jb, s, :],
                                    at[s][:, jb * N + ic * NQ: jb * N + (ic + 1) * NQ],
                                    start=(k == 0), stop=False,
                                    skip_group_check=True)
                                k += 1
                    nc.tensor.matmul(
                        p1[:], w0ru[:], x0Tb[:, ic * NQ:(ic + 1) * NQ],
                        start=False, stop=True, skip_group_check=True)
                    nc.scalar.activation(
                        rT[:, ic * NQ:(ic + 1) * NQ], p1[0:U, :],
                        mybir.ActivationFunctionType.Sigmoid, bias=bias["r"][:])
                    nc.scalar.activation(
                        uT[:, ic * NQ:(ic + 1) * NQ], p1[U:2 * U, :],
                        mybir.ActivationFunctionType.Sigmoid, bias=bias["u"][:])

                x0cT = actp.tile([F, N], BF16, tag="x0cT", name="x0cT")
                zc = actp.tile([P, JB * S * U], BF16, tag="zc", name="zc")
                WC = S * U

                def zc_half(hf):
                    # Z_c for both supports in one 128-col moving sweep per
                    # node block of this column half; x0c^T = [(r*H)^T|x^T]
                    sl = slice(hf * NQ, (hf + 1) * NQ)
                    if hf == 0:
                        nc.vector.tensor_copy(x0cT[U:F, :], x0Tb[U:F, :])
                    nc.vector.tensor_mul(
                        x0cT[0:U, sl], rT[:, sl], x0Tb[0:U, sl])
                    pz = psM.tile([P, 4 * WC], F32, tag="psM", name="pzc")
                    for q in range(4 * hf, 4 * hf + 4):
                        nc.tensor.matmul(
                            pz[:, (q - 4 * hf) * WC:(q - 4 * hf + 1) * WC],
                            x0cT[:, q * P:(q + 1) * P],
                            wc_cat[:], start=(q == 4 * hf),
                            stop=(q == 4 * hf + 3))
                    nc.scalar.copy(
                        zc[:, 4 * hf * WC:(4 * hf + 4) * WC], pz[:])

                def finish_ru():
                    # g1 = u*H^T and w = 1-u as soon as u is complete; they
                    # are DVE ops and unblock the tail's h math
                    g1 = actp.tile([U, N], F32, tag="g1", name="g1")
                    nc.vector.tensor_mul(g1[:], uT[:], x0Tb[0:U, :])
                    wT = actp.tile([U, N], F32, tag="wT", name="wT")
                    nc.vector.tensor_scalar(wT[:], uT[:], -1.0, 1.0,
                                            mybir.AluOpType.mult,
                                            mybir.AluOpType.add)
                    st["g1"], st["wT"] = g1, wT

                st["rT"], st["uT"] = rT, uT
                st["phase1"], st["finish_ru"] = phase1, finish_ru
                st["x0cT"], st["zc"], st["zc_half"] = x0cT, zc, zc_half

            def tail(b):
                st = state[b]
                at = st["at"]
                x0cT, zc = st["x0cT"], st["zc"]
                g1, wT = st["g1"], st["wT"]
                x0Tb = x0Tb_l[b]

                # phase 2 + h, pipelined per column half; the j-blocks of
                # the first zc half accumulate first so the second half's
                # zc matmuls overlap the p2 accumulation
                zcv = zc[:].rearrange("p (jb s m) -> p jb s m", s=S, m=U)

                def p2_half(p2, ic, hf, k0):
                    k = k0
                    for s in range(S):
                        for jb in range(4 * hf, 4 * hf + 4):
                            nc.tensor.matmul(
                                p2[:],
                                zcv[:, jb, s, :],
                                at[s][:, jb * N + ic * NQ: jb * N + (ic + 1) * NQ],
                                start=(k == 0), stop=False,
                                skip_group_check=True)
                            k += 1
                    return k

                cT = actp.tile([U, N], F32, tag="cT", name="cT")
                hTb = actp.tile([U, N], BF16, tag="hTb", name="hTb")
                hnat = actp.tile([P, JB * U], F32, tag="hnat", name="hnat")
                p2s = [psB.tile([U, NQ], F32, tag="psB", name=f"p2_{ic}")
                       for ic in range(NIC)]
                # half0 of ic=0 first (zc half0 ready), then the second zc
                # half (gpsimd-free: DVE mul + PE matmuls), then the rest
                k00 = p2_half(p2s[0], 0, 0, 0)
                st["zc_half"](1)
                for ic in range(NIC):
                    p2 = p2s[ic]
                    if ic == 0:
                        k = p2_half(p2, ic, 1, k00)
                    else:
                        k = p2_half(p2, ic, 0, 0)
                        k = p2_half(p2, ic, 1, k)
                    nc.tensor.matmul(
                        p2[:], wc0[:], x0cT[:, ic * NQ:(ic + 1) * NQ],
                        start=False, stop=True, skip_group_check=True)
                    sl = slice(ic * NQ, (ic + 1) * NQ)
                    nc.scalar.activation(
                        cT[:, sl], p2[:],
                        mybir.ActivationFunctionType.Tanh, bias=bias["c"][:])
                    # h^T = c^T * (1-u^T) + u^T * H^T
                    nc.vector.tensor_mul(cT[:, sl], cT[:, sl], wT[:, sl])
                    nc.vector.tensor_add(hTb[:, sl], cT[:, sl], g1[:, sl])
                    ph = psM.tile([P, 2 * JB * U], BF16, tag="psM", name="ph")
                    for jb in range(4 * ic, 4 * ic + 4):
                        nc.tensor.matmul(
                            ph[:, (jb - 4 * ic) * U:(jb - 4 * ic + 1) * U],
                            hTb[:, jb * P:(jb + 1) * P],
                            id_bf[0:U, 0:U],
                            start=(jb == 4 * ic), stop=(jb == 4 * ic + 3),
                            is_transpose=True)
                    hh = slice(ic * 4 * U, (ic + 1) * 4 * U)
                    nc.scalar.copy(hnat[:, hh], ph[:, 0:4 * U])
                    nc.sync.dma_start(
                        t_out.ap()[b].rearrange("(p m) -> p m", p=P)[:, hh],
                        hnat[:, hh])

            # ---- staged main loop ----
            # PE order: first chunk's transposes as soon as data lands,
            # prologues threaded between transpose chunks, loads for b+1
            # issued before the tail of b so SWDGE never waits on compute.
            # phase/tail work is interleaved between transpose chunks so a
            # not-yet-landed chunk never blocks ready work at the PE queue
            # head.
            transpose_chunk(0, 0)
            prologue(0)
            prologue(1)
            transpose_chunk(0, 1)
            prologue(2)
            prologue(3)
            prepare_phase1(0)
            state[0]["phase1"](0)
            state[0]["zc_half"](0)
            transpose_chunk(0, 2)
            transpose_chunk(0, 3)
            state[0]["phase1"](1)
            state[0]["finish_ru"]()
            for b in range(1, BPC):
                issue_loads(b, range(NCH))
                transpose_chunk(b, 0)
                transpose_chunk(b, 1)
                tail(b - 1)
                transpose_chunk(b, 2)
                prepare_phase1(b)
                state[b]["phase1"](0)
                state[b]["zc_half"](0)
                transpose_chunk(b, 3)
                state[b]["phase1"](1)
                state[b]["finish_ru"]()
            tail(BPC - 1)

    nc.finalize()
    return nc


def _make_in_maps(inputs):
    in_maps = []
    for c in range(NCORES):
        lo, hi = c * BPC, (c + 1) * BPC
        in_maps.append({
            "inputs": np.ascontiguousarray(inputs["inputs"][lo:hi], np.float32),
            "supports": np.ascontiguousarray(inputs["supports"][lo:hi], np.float32),
            "h_prev": np.ascontiguousarray(inputs["h_prev"][lo:hi], np.float32),
            "r_kernel": np.ascontiguousarray(inputs["r_kernel"], np.float32),
            "u_kernel": np.ascontiguousarray(inputs["u_kernel"], np.float32),
            "c_kernel": np.ascontiguousarray(inputs["c_kernel"], np.float32),
            "r_bias": np.ascontiguousarray(inputs["r_bias"], np.float32),
            "u_bias": np.ascontiguousarray(inputs["u_bias"], np.float32),
            "c_bias": np.ascontiguousarray(inputs["c_bias"], np.float32),
        })
    return in_maps


def kernel(**inputs):
    nc = _COMPILED.get("nc")
    if nc is None:
        nc = _COMPILED["nc"] = _build()

    in_maps = _make_in_maps(inputs)
    last_err = None
    for _ in range(3):
        try:
            res = run_bass_kernel_spmd(nc, in_maps, core_ids=list(range(NCORES)))
            out = np.concatenate(
                [np.asarray(res.results[c]["out"]) for c in range(NCORES)], axis=0)
            return out.astype(np.float32)
        except Exception as e:  # sporadic NRT_EXEC_UNIT_UNRECOVERABLE flakes
            last_err = e
    raise last_err


# revision 19
# speedup vs baseline: 1.0666x; 1.0169x over previous
"""GCGRU cell (graph-conv GRU, diffusion-conv gates) on 8 TRN2 NeuronCores.

Math (per batch b, N=1024 nodes, D=2 in-feats, U=64 units, S=2 supports):
  x0   = [H_b | inputs_b]                          (N, 66)  (feature-permuted)
  for gate g in {r, u, c}:
    pre_g = x0g @ Wg_m0 + sum_s A_s @ (x0g @ Wg_{m=s+1}) + bias_g
  (reassociated: (A_s @ x0) @ W == A_s @ (x0 @ W), so the N x N supports
   multiply a tiny (N, 64) matrix instead of the other association order)
  r, u = sigmoid(pre_r), sigmoid(pre_u); c = tanh(pre_c with x0c=[r*H|inputs])
  h = u * H + (1 - u) * c

Implementation notes:
  - Data parallel over batch: 32 batches -> 4 per core, no collectives.
  - supports[b] is cast f32->bf16 *during* the HBM->SBUF DMA (SWDGE cast)
    in natural layout (i on partitions, (j,s) free).
  - The j-contraction needs j on partitions, so A is transposed 128x128
    tile-wise on the TensorEngine (transpose-mode matmul with a bf16
    identity); the bf16 PSUM tiles drain on the DVE with an fp8e4 cast,
    so A^T lives in SBUF as fp8 (halving its footprint).
  - r/u pre-activations accumulate with fp8 DoubleRow matmuls (two
    j-blocks per instruction, Z_ru also fp8); the c pass runs mixed
    (fp8 A^T moving x bf16 Z_c stationary).  Tolerance is 2e-2 and the
    fp8 path lands ~1.7e-2 (validated against the fp32 reference).
  - h_prev/inputs load contiguously (2KB runs, in-DMA bf16 cast) in a
    node%8-interleaved partition layout; the PE transposes it and the
    PSUM drain un-interleaves with a strided write AP.
  - The supports chunk loads are issued FIRST (ahead of h/x) and batch
    0's first chunk is split so the PE starts transposing ~1.5us in.
    Gate elementwise runs on the DVE/Scalar so the GpSimd queue carries
    only SWDGE descriptor generation and never blocks the A stream.
"""

import numpy as np

import concourse.bacc as bacc
import concourse.mybir as mybir
import concourse.tile as tile
from concourse.bass_utils import run_bass_kernel_spmd
from concourse.masks import make_identity

B, N, D, U, S = 32, 1024, 2, 64, 2
F = D + U                      # 66
NCORES = 8
BPC = B // NCORES              # 4 batches per core
P = 128                        # partitions
JB = N // P                    # 8 j-blocks per support
K8 = N // P                    # 8 nodes per partition in contiguous layout
F32 = mybir.dt.float32
BF16 = mybir.dt.bfloat16
F8 = mybir.dt.float8e4

FP8 = True                     # fp8e4 A^T / Z_ru fast path

_COMPILED = {}


def _build():
    nc = bacc.Bacc("TRN2", target_bir_lowering=False, debug=False)

    t_inputs = nc.dram_tensor("inputs", [BPC, N, D], F32, kind="ExternalInput")
    t_supports = nc.dram_tensor("supports", [BPC, N, N, S], F32, kind="ExternalInput")
    t_hprev = nc.dram_tensor("h_prev", [BPC, N * U], F32, kind="ExternalInput")
    t_wk = {g: nc.dram_tensor(f"{g}_kernel", [F * 3, U], F32, kind="ExternalInput")
            for g in "ruc"}
    t_wb = {g: nc.dram_tensor(f"{g}_bias", [U], F32, kind="ExternalInput")
            for g in "ruc"}
    t_out = nc.dram_tensor("out", [BPC, N * U], F32, kind="ExternalOutput")

    QC = 2                 # i-tiles per load chunk
    NCH = N // (QC * P)    # 4 chunks per batch
    NQ = 512               # phase moving-slice width (one PSUM bank of f32)
    NIC = N // NQ          # 2 phase column-groups per batch
    AT_DT = F8 if FP8 else BF16
    AB_DT = F8 if FP8 else BF16
    DR = mybir.MatmulPerfMode.DoubleRow

    with tile.TileContext(nc) as tc:
        with (
            tc.tile_pool(name="const", bufs=1) as constp,
            tc.tile_pool(name="wt", bufs=1) as wtp,
            tc.tile_pool(name="pre", bufs=BPC) as prep,
            tc.tile_pool(name="abf", bufs=8) as abfp,
            tc.tile_pool(name="at", bufs=2) as atp,
            tc.tile_pool(name="act", bufs=2) as actp,
            tc.tile_pool(name="psT", bufs=2, space="PSUM") as psT,
            tc.tile_pool(name="psB", bufs=2, space="PSUM") as psB,
            tc.tile_pool(name="psM", bufs=2, space="PSUM") as psM,
        ):
            sup4 = t_supports.ap().rearrange(
                "b (p q) j two -> b p q (j two)", q=K8)

            # ---- constants ----
            id_bf = constp.tile([P, P], BF16, tag="id_bf")
            make_identity(nc, id_bf[:])
            if FP8:
                id_ab = constp.tile([P, P], F8, tag="id_ab")
                nc.vector.tensor_copy(id_ab[:], id_bf[:])
            else:
                id_ab = id_bf

            # ---- per-batch state for the staged main loop ----
            state = {}

            def issue_loads(b, chs, split=False):
                if b not in state:
                    at = [atp.tile([P, JB * N], AT_DT, tag=f"at{s}",
                                   name=f"at{s}") for s in range(S)]
                    state[b] = {"at": at, "abts": [None] * NCH}
                for ch in chs:
                    ab = abfp.tile([P, QC * N * S], AB_DT, tag="abf", name="ab")
                    if split:
                        ab4 = ab[:].rearrange("p (q m) -> p q m", q=QC)
                        for q in range(QC):
                            nc.gpsimd.dma_start(
                                ab4[:, q, :], sup4[b, :, ch * QC + q, :])
                    else:
                        nc.gpsimd.dma_start(
                            ab[:], sup4[b, :, ch * QC:(ch + 1) * QC, :])
                    state[b]["abts"][ch] = ab

            # first sub-chunk of the whole kernel goes out before h/x so
            # the PE has transpose work as early as possible; h/x slot in
            # after it so the prologue can fill the wait for sub-chunk 2
            issue_loads(0, [])
            ab00 = abfp.tile([P, QC * N * S], AB_DT, tag="abf", name="ab")
            ab00q = ab00[:].rearrange("p (q m) -> p q m", q=QC)
            nc.gpsimd.dma_start(ab00q[:, 0, :], sup4[0, :, 0, :])
            state[0]["abts"][0] = ab00

            # ---- gate weights, hop blocks, permuted to [H|inputs], bf16 ----
            # W rows are (f, m) pairs, m fastest: row f*3 + m.  One staging
            # DMA per gate (rows permuted to [H|inputs]), bf16 casts on DVE.
            wst = {}
            for g in "ruc":
                st = wtp.tile([F, 3 * U], F32, tag=f"wst_{g}", name=f"wst_{g}")
                src = t_wk[g].ap().rearrange("(f three) u -> f (three u)", three=3)
                nc.sync.dma_start(st[0:U, :], src[D:F, :])
                nc.sync.dma_start(st[U:F, :], src[0:D, :])
                wst[g] = st

            def w_block(g, m):
                return wst[g][:, m * U:(m + 1) * U]

            w0ru = wtp.tile([F, 2 * U], BF16, tag="w0ru")
            nc.vector.tensor_copy(w0ru[:, 0:U], w_block("r", 0))
            nc.vector.tensor_copy(w0ru[:, U:2 * U], w_block("u", 0))
            wru_cat = wtp.tile([F, S * 2 * U], BF16, tag="wru_cat")
            for s in range(S):
                nc.vector.tensor_copy(
                    wru_cat[:, s * 2 * U:s * 2 * U + U], w_block("r", s + 1))
                nc.vector.tensor_copy(
                    wru_cat[:, s * 2 * U + U:(s + 1) * 2 * U],
                    w_block("u", s + 1))
            wc0 = wtp.tile([F, U], BF16, tag="wc0")
            nc.vector.tensor_copy(wc0[:], w_block("c", 0))
            wc_cat = wtp.tile([F, S * U], BF16, tag="wc_cat")
            for s in range(S):
                nc.vector.tensor_copy(
                    wc_cat[:, s * U:(s + 1) * U], w_block("c", s + 1))

            bias = {}
            for g in "ruc":
                bt = wtp.tile([U, 1], F32, tag=f"bias_{g}")
                nc.sync.dma_start(bt[:], t_wb[g].ap().rearrange("(u one) -> u one", one=1))
                bias[g] = bt

            # ---- prologue: x0^T and Z_ru for ALL batches ----
            # h_prev/inputs load contiguously (node n = 8p+k on partition p,
            # slot k) with in-DMA bf16 cast; 16 transpose-mode matmuls per
            # batch land [H^T | x^T] interleaved in one PSUM bank; the
            # scalar drain un-interleaves via a strided write AP.
            hcall = prep.tile([P, BPC * K8 * U], BF16, tag="hcb", name="hcb",
                              bufs=1)
            nc.gpsimd.dma_start(
                hcall[:].rearrange("p (b m) -> p b m", b=BPC),
                t_hprev.ap().rearrange("b (p m) -> p b m", p=P))
            xcall = prep.tile([P, BPC * K8 * D], BF16, tag="xcb", name="xcb",
                              bufs=1)
            nc.gpsimd.dma_start(
                xcall[:].rearrange("p (b m) -> p b m", b=BPC),
                t_inputs.ap().rearrange("b (p k) d -> p b (k d)", p=P))
            hcb = [hcall[:, b * K8 * U:(b + 1) * K8 * U] for b in range(BPC)]
            xcb = [xcall[:, b * K8 * D:(b + 1) * K8 * D] for b in range(BPC)]

            # second sub-chunk and the rest of batch 0 behind h/x
            nc.gpsimd.dma_start(ab00q[:, 1, :], sup4[0, :, 1, :])
            issue_loads(0, [1, 2, 3])

            x0Tb_l, zru_l = [], []

            def prologue(b):
                px = psM.tile([F, JB * P], BF16, tag="psM", name="px")
                for k in range(K8):
                    nc.tensor.matmul(
                        px[0:U, k * P:(k + 1) * P],
                        hcb[b][:, k * U:(k + 1) * U],
                        id_bf[:], start=(k == 0), stop=False,
                        is_transpose=True)
                for k in range(K8):
                    nc.tensor.matmul(
                        px[U:F, k * P:(k + 1) * P],
                        xcb[b][:, k * D:(k + 1) * D],
                        id_bf[:], start=False, stop=(k == K8 - 1),
                        is_transpose=True)
                x0Tb = prep.tile([F, N], BF16, tag="x0Tb", name="x0Tb")
                nc.scalar.copy(x0Tb[:], px[:])
                x0Tb_l.append(x0Tb)

                # Z_ru for both supports in one 256-col moving sweep per
                # node block; layout (jb, s, u2) so the copy is contiguous
                z = prep.tile([P, JB * S * 2 * U], AT_DT, tag="zru",
                              name="zru")
                W2 = S * 2 * U
                for qp in range(4):
                    pz = psM.tile([P, 2 * W2], F32, tag="psM", name="pz")
                    for q in (2 * qp, 2 * qp + 1):
                        nc.tensor.matmul(
                            pz[:, (q - 2 * qp) * W2:(q - 2 * qp + 1) * W2],
                            x0Tb[:, q * P:(q + 1) * P],
                            wru_cat[:], start=(q == 2 * qp),
                            stop=(q == 2 * qp + 1))
                    nc.scalar.copy(
                        z[:, 2 * qp * W2:(2 * qp + 2) * W2], pz[:])
                zru_l.append(z)

            def transpose_chunk(b, ch):
                # fp8 transpose-mode requires element step 2 in PSUM:
                # allocate double-width, write every other byte, and the
                # drains read the strided view.  Both q sub-tiles of a
                # chunk share one two-bank PSUM tile so the drain is a
                # single [128, 2*128-col] copy per (s, chunk); s=0 drains
                # on the DVE and s=1 on the Scalar engine to split the
                # PSUM-recycle bandwidth.
                st = state[b]
                at, abts = st["at"], st["abts"]
                ab5 = abts[ch][:].rearrange(
                    "p (q g e two) -> p q e g two", q=QC, e=K8, two=2)
                for s in range(S):
                    atv = at[s][:].rearrange("p (jb n) -> p jb n", n=N)
                    if FP8:
                        ptb = psT.tile([P, QC * 2 * JB * P], AB_DT,
                                       tag="psT", name="pt")
                        ptq = ptb[:].rearrange(
                            "p (q n two) -> p q n two", q=QC, two=2)
                        pts = [ptq[:, q, :, 0] for q in range(QC)]
                        # drain source ordered as dest: [jb, q, i]
                        dr_src = ptb[:].rearrange(
                            "p (q jb i two) -> p jb q (i two)",
                            q=QC, jb=JB, two=2)[:, :, :, 0:2 * P:2]
                    else:
                        ptb = psT.tile([P, QC * JB * P], AB_DT, tag="psT",
                                       name="pt")
                        ptq = ptb[:].rearrange(
                            "p (q n) -> p q n", q=QC)
                        pts = [ptq[:, q, :] for q in range(QC)]
                        dr_src = ptb[:].rearrange(
                            "p (q jb i) -> p jb q i", q=QC, jb=JB)
                    for q in range(QC):
                        pt = pts[q]
                        for e in range(JB):
                            nc.tensor.matmul(
                                pt[:, e * P:(e + 1) * P],
                                ab5[:, q, e, :, s],
                                id_ab[:],
                                start=(e == 0), stop=(e == JB - 1),
                                is_transpose=True)
                    c0 = ch * QC * P
                    if s == 0:
                        nc.vector.tensor_copy(
                            atv[:, :, c0:c0 + QC * P], dr_src)
                    else:
                        nc.scalar.copy(
                            atv[:, :, c0:c0 + QC * P], dr_src)

            def prepare_phase1(b):
                st = state[b]
                at = st["at"]
                x0Tb, zru = x0Tb_l[b], zru_l[b]

                rT = actp.tile([U, N], BF16, tag="rT", name="rT")
                uT = actp.tile([U, N], F32, tag="uT", name="uT")

                zrv = zru[:].rearrange(
                    "p (jb s m) -> p jb s m", s=S, m=2 * U)

                def phase1(ic):
                    p1 = psB.tile([P, NQ], F32, tag="psB", name="p1")
                    k = 0
                    if FP8:
                        for s in range(S):
                            atv = at[s][:].rearrange(
                                "p (jb n) -> p jb n", n=N)
                            for jp in range(JB // 2):
                                nc.tensor.matmul(
                                    p1[:],
                                    zrv[:, 2 * jp:2 * jp + 2, s, :],
                                    atv[:, 2 * jp:2 * jp + 2,
                                        ic * NQ:(ic + 1) * NQ],
                                    start=(k == 0), stop=False,
                                    perf_mode=DR, skip_group_check=True)
                                k += 1
                    else:
                        for s in range(S):
                            for jb in range(JB):
                                nc.tensor.matmul(
                                    p1[:],
                                    zrv[:, jb, s, :],
                                    at[s][:, jb * N + ic * NQ: jb * N + (ic + 1) * NQ],
                                    start=(k == 0), stop=False,
                                    skip_group_check=True)
                                k += 1
                    nc.tensor.matmul(
                        p1[:], w0ru[:], x0Tb[:, ic * NQ:(ic + 1) * NQ],
                        start=False, stop=True, skip_group_check=True)
                    nc.scalar.activation(
                        rT[:, ic * NQ:(ic + 1) * NQ], p1[0:U, :],
                        mybir.ActivationFunctionType.Sigmoid, bias=bias["r"][:])
                    nc.scalar.activation(
                        uT[:, ic * NQ:(ic + 1) * NQ], p1[U:2 * U, :],
                        mybir.ActivationFunctionType.Sigmoid, bias=bias["u"][:])

                x0cT = actp.tile([F, N], BF16, tag="x0cT", name="x0cT")
                zc = actp.tile([P, JB * S * U], BF16, tag="zc", name="zc")
                WC = S * U

                def zc_half(hf):
                    # Z_c for both supports in one 128-col moving sweep per
                    # node block of this column half; x0c^T = [(r*H)^T|x^T]
                    sl = slice(hf * NQ, (hf + 1) * NQ)
                    if hf == 0:
                        nc.vector.tensor_copy(x0cT[U:F, :], x0Tb[U:F, :])
                    nc.vector.tensor_mul(
                        x0cT[0:U, sl], rT[:, sl], x0Tb[0:U, sl])
                    pz = psM.tile([P, 4 * WC], F32, tag="psM", name="pzc")
                    for q in range(4 * hf, 4 * hf + 4):
                        nc.tensor.matmul(
                            pz[:, (q - 4 * hf) * WC:(q - 4 * hf + 1) * WC],
                            x0cT[:, q * P:(q + 1) * P],
                            wc_cat[:], start=(q == 4 * hf),
                            stop=(q == 4 * hf + 3))
                    nc.scalar.copy(
                        zc[:, 4 * hf * WC:(4 * hf + 4) * WC], pz[:])

                def finish_ru():
                    # g1 = u*H^T and w = 1-u as soon as u is complete; they
                    # are DVE ops and unblock the tail's h math
                    g1 = actp.tile([U, N], F32, tag="g1", name="g1")
                    nc.vector.tensor_mul(g1[:], uT[:], x0Tb[0:U, :])
                    wT = actp.tile([U, N], F32, tag="wT", name="wT")
                    nc.vector.tensor_scalar(wT[:], uT[:], -1.0, 1.0,
                                            mybir.AluOpType.mult,
                                            mybir.AluOpType.add)
                    st["g1"], st["wT"] = g1, wT

                st["rT"], st["uT"] = rT, uT
                st["phase1"], st["finish_ru"] = phase1, finish_ru
                st["x0cT"], st["zc"], st["zc_half"] = x0cT, zc, zc_half

            def tail(b):
                st = state[b]
                at = st["at"]
                x0cT, zc = st["x0cT"], st["zc"]
                g1, wT = st["g1"], st["wT"]
                x0Tb = x0Tb_l[b]

                # phase 2 + h, pipelined per column half; the j-blocks of
                # the first zc half accumulate first so the second half's
                # zc matmuls overlap the p2 accumulation
                zcv = zc[:].rearrange("p (jb s m) -> p jb s m", s=S, m=U)

                def p2_half(p2, ic, hf, k0):
                    k = k0
                    for s in range(S):
                        for jb in range(4 * hf, 4 * hf + 4):
                            nc.tensor.matmul(
                                p2[:],
                                zcv[:, jb, s, :],
                                at[s][:, jb * N + ic * NQ: jb * N + (ic + 1) * NQ],
                                start=(k == 0), stop=False,
                                skip_group_check=True)
                            k += 1
                    return k

                cT = actp.tile([U, N], F32, tag="cT", name="cT")
                hTb = actp.tile([U, N], BF16, tag="hTb", name="hTb")
                hnat = actp.tile([P, JB * U], F32, tag="hnat", name="hnat")
                p2s = [psB.tile([U, NQ], F32, tag="psB", name=f"p2_{ic}")
                       for ic in range(NIC)]
                # half0 of ic=0 first (zc half0 ready), then the second zc
                # half (gpsimd-free: DVE mul + PE matmuls), then the rest
                k00 = p2_half(p2s[0], 0, 0, 0)
                st["zc_half"](1)
                for ic in range(NIC):
                    p2 = p2s[ic]
                    if ic == 0:
                        k = p2_half(p2, ic, 1, k00)
                    else:
                        k = p2_half(p2, ic, 0, 0)
                        k = p2_half(p2, ic, 1, k)
                    nc.tensor.matmul(
                        p2[:], wc0[:], x0cT[:, ic * NQ:(ic + 1) * NQ],
                        start=False, stop=True, skip_group_check=True)
                    # h^T = c^T * (1-u^T) + u^T * H^T, in 256-col quarters
                    # so the h transposes start while the second quarter's
                    # tanh/mul/add still runs
                    HQ = NQ // 2
                    ph = psM.tile([P, 2 * JB * U], BF16, tag="psM", name="ph")
                    for qq in range(2):
                        sq = slice(ic * NQ + qq * HQ, ic * NQ + (qq + 1) * HQ)
                        nc.scalar.activation(
                            cT[:, sq], p2[:, qq * HQ:(qq + 1) * HQ],
                            mybir.ActivationFunctionType.Tanh,
                            bias=bias["c"][:])
                        nc.vector.tensor_mul(cT[:, sq], cT[:, sq], wT[:, sq])
                        nc.vector.tensor_add(hTb[:, sq], cT[:, sq], g1[:, sq])
                        jb0 = 4 * ic + 2 * qq
                        for jb in (jb0, jb0 + 1):
                            nc.tensor.matmul(
                                ph[:, (jb - 4 * ic) * U:(jb - 4 * ic + 1) * U],
                                hTb[:, jb * P:(jb + 1) * P],
                                id_bf[0:U, 0:U],
                                start=(jb == jb0), stop=(jb == jb0 + 1),
                                is_transpose=True)
                    hh = slice(ic * 4 * U, (ic + 1) * 4 * U)
                    nc.scalar.copy(hnat[:, hh], ph[:, 0:4 * U])
                    nc.sync.dma_start(
                        t_out.ap()[b].rearrange("(p m) -> p m", p=P)[:, hh],
                        hnat[:, hh])

            # ---- staged main loop ----
            # PE order: first chunk's transposes as soon as data lands,
            # prologues threaded between transpose chunks, loads for b+1
            # issued before the tail of b so SWDGE never waits on compute.
            # phase/tail work is interleaved between transpose chunks so a
            # not-yet-landed chunk never blocks ready work at the PE queue
            # head.
            transpose_chunk(0, 0)
            prologue(0)
            prologue(1)
            transpose_chunk(0, 1)
            prologue(2)
            prologue(3)
            prepare_phase1(0)
            state[0]["phase1"](0)
            state[0]["zc_half"](0)
            transpose_chunk(0, 2)
            transpose_chunk(0, 3)
            state[0]["phase1"](1)
            state[0]["finish_ru"]()
            for b in range(1, BPC):
                issue_loads(b, range(NCH))
                transpose_chunk(b, 0)
                transpose_chunk(b, 1)
                tail(b - 1)
                transpose_chunk(b, 2)
                prepare_phase1(b)
                state[b]["phase1"](0)
                state[b]["zc_half"](0)
                transpose_chunk(b, 3)
                state[b]["phase1"](1)
                state[b]["finish_ru"]()
            tail(BPC - 1)

    nc.finalize()
    return nc


def _make_in_maps(inputs):
    in_maps = []
    for c in range(NCORES):
        lo, hi = c * BPC, (c + 1) * BPC
        in_maps.append({
            "inputs": np.ascontiguousarray(inputs["inputs"][lo:hi], np.float32),
            "supports": np.ascontiguousarray(inputs["supports"][lo:hi], np.float32),
            "h_prev": np.ascontiguousarray(inputs["h_prev"][lo:hi], np.float32),
            "r_kernel": np.ascontiguousarray(inputs["r_kernel"], np.float32),
            "u_kernel": np.ascontiguousarray(inputs["u_kernel"], np.float32),
            "c_kernel": np.ascontiguousarray(inputs["c_kernel"], np.float32),
            "r_bias": np.ascontiguousarray(inputs["r_bias"], np.float32),
            "u_bias": np.ascontiguousarray(inputs["u_bias"], np.float32),
            "c_bias": np.ascontiguousarray(inputs["c_bias"], np.float32),
        })
    return in_maps


def kernel(**inputs):
    nc = _COMPILED.get("nc")
    if nc is None:
        nc = _COMPILED["nc"] = _build()

    in_maps = _make_in_maps(inputs)
    last_err = None
    for _ in range(3):
        try:
            res = run_bass_kernel_spmd(nc, in_maps, core_ids=list(range(NCORES)))
            out = np.concatenate(
                [np.asarray(res.results[c]["out"]) for c in range(NCORES)], axis=0)
            return out.astype(np.float32)
        except Exception as e:  # sporadic NRT_EXEC_UNIT_UNRECOVERABLE flakes
            last_err = e
    raise last_err
